# revision 1
# baseline (speedup 1.0000x reference)
"""Causal self-attention with ALiBi for Trainium2, sharded over 8 NeuronCores.

Problem: B=2, T=2048, C=1024, H=16 heads, D=64. y = proj(softmax(qk^T/8 + alibi) v).

Sharding (per spec hint): data-parallel on B x tensor-parallel on heads.
Core c handles batch b = c // 4 and the 4 heads [4*(c%4), 4*(c%4)+4).
Each core computes its heads' attention output and a partial projection
(contracting only its 256 columns of w_proj); the host sums the 4 partials
per batch.

Host-side prep (not device work): x is pre-transposed to xT=(C,T) per batch,
weights are pre-sliced/transposed per core so the device kernel needs no
on-chip transposes. The attention scale 1/8 is folded into wq.

Device pipeline per core (all matmuls in float32r = fp22, full PE rate):
  1. qT/kT = W^T-slices @ xT     -> (64, T) per head, feature-major ("transposed")
  2. v     = x @ Wv^T            -> (T, 256) natural, with a ones column
     appended per head (gives the softmax denominator for free).
  3. ALiBi via 2 extra contraction rows: k-side [j; 1], q-side
     [slope; -slope*i] => s_T[j,i] = q.k/8 + slope*(j-i), K=66.
  4. e_T = exp(s_T) on ACT; causal mask applied by zeroing e_T's upper
     triangle on GpSimd (affine_select) for diagonal-crossing tiles only.
  5. y_aug^T = [v | 1]^T @ e_T accumulated over Tk tiles -> rows 0:64
     unnormalized y^T, row 64 the denominator.
  6. normalize via batched reciprocal + partition-broadcast multiply.
  7. partial out = y^T.T @ wp^T-slice, DMA to DRAM.

DVE/ACT engines are partition-locked (operands must share the start
partition), so moving a head's 64 rows from psum partitions 64:128 down to
0:64 goes through a small SBUF->SBUF DMA (staging tile) instead.
"""

import math

import numpy as np

B, T, C = 2, 2048, 1024
H, D = 16, 64
HL = 4          # heads per core
N_CORES = 8
P = 128         # partitions
CS = 512        # Tq chunk (matmul moving dim)
CI = C // P     # 8 contraction chunks
TT = T // P     # 16 T tiles
NQ = T // CS    # 4 Tq chunks

_BUILT = {}


def _alibi_slopes(n_heads):
    start = 2.0 ** (-(2.0 ** (-(math.log2(n_heads) - 3))))
    return np.array([start * start**i for i in range(n_heads)], dtype=np.float32)


def _build():
    """Build + compile the (single, SPMD) Bass module. Cached per process."""
    if "nc" in _BUILT:
        return _BUILT["nc"]

    from contextlib import ExitStack

    import concourse.bacc as bacc
    import concourse.mybir as mybir
    import concourse.tile as tile

    f32 = mybir.dt.float32
    f32r = mybir.dt.float32r
    EXP = mybir.ActivationFunctionType.Exp
    GE = mybir.AluOpType.is_ge

    nc = bacc.Bacc("TRN2", target_bir_lowering=False)

    xT = nc.dram_tensor("xT", [C, T], f32, kind="ExternalInput").ap()
    wqT = nc.dram_tensor("wqT", [C, HL * D], f32, kind="ExternalInput").ap()
    wkT = nc.dram_tensor("wkT", [C, HL * D], f32, kind="ExternalInput").ap()
    wvT = nc.dram_tensor("wvT", [C, HL * D], f32, kind="ExternalInput").ap()
    wpT = nc.dram_tensor("wpT", [HL * D, C], f32, kind="ExternalInput").ap()
    kaug = nc.dram_tensor("kaug", [2, T], f32, kind="ExternalInput").ap()
    qaug = nc.dram_tensor("qaug", [HL, 2, T], f32, kind="ExternalInput").ap()
    vones = nc.dram_tensor("vones", [P, HL], f32, kind="ExternalInput").ap()
    outp = nc.dram_tensor("outp", [T, C], f32, kind="ExternalOutput").ap()

    def mm(out, lhsT, rhs, start, stop):
        nc.tensor.matmul(out, lhsT.bitcast(f32r), rhs.bitcast(f32r),
                         start=start, stop=stop)

    def r(ap):
        # walrus requires every writer of an fp32r-matmul operand to declare
        # fp32r output; the PE truncates to fp22 on read either way.
        return ap.bitcast(f32r)

    with tile.TileContext(nc) as tc, ExitStack() as ctx:
        xp = ctx.enter_context(tc.tile_pool(name="xp", bufs=1))
        wpool = ctx.enter_context(tc.tile_pool(name="wpool", bufs=1))
        vp = ctx.enter_context(tc.tile_pool(name="vp", bufs=1))
        kqp = ctx.enter_context(tc.tile_pool(name="kqp", bufs=2))
        ep = ctx.enter_context(tc.tile_pool(name="ep", bufs=4))
        yp = ctx.enter_context(tc.tile_pool(name="yp", bufs=1))
        mp = ctx.enter_context(tc.tile_pool(name="mp", bufs=2))
        op_pool = ctx.enter_context(tc.tile_pool(name="op", bufs=2))
        ps_mm = ctx.enter_context(tc.tile_pool(name="ps_mm", bufs=2, space="PSUM"))
        ps_s = ctx.enter_context(tc.tile_pool(name="ps_s", bufs=2, space="PSUM"))
        ps_y = ctx.enter_context(tc.tile_pool(name="ps_y", bufs=2, space="PSUM"))

        # ---- resident loads: wq/wk/x interleaved per chunk so the qk
        #      accumulation can start as soon as chunk 0 lands; wv/wp deferred.
        w_sb = {}
        x_sb = []
        for ci in range(CI):
            for nm, srct in (("q", wqT), ("k", wkT)):
                t = wpool.tile([P, HL * D], f32, name=f"w{nm}{ci}", tag=f"w{nm}{ci}")
                nc.sync.dma_start(r(t), r(srct[ci * P:(ci + 1) * P, :]))
                w_sb[nm, ci] = t
            t = xp.tile([P, T], f32, name=f"x{ci}", tag=f"x{ci}")
            nc.sync.dma_start(r(t), r(xT[ci * P:(ci + 1) * P, :]))
            x_sb.append(t)

        yT_sb = [yp.tile([P, T], f32, name=f"yT{i}", tag=f"yT{i}") for i in range(2)]
        denom_sb = [mp.tile([NQ, CS], f32, name=f"denom{i}", tag=f"denom{i}",
                            bufs=1) for i in range(HL)]

        v_sb = []

        def sel(ap, n, base):
            # zero the causally-masked region: keep where free - part + base >= 0
            nc.gpsimd.affine_select(out=r(ap), in_=r(ap), compare_op=GE,
                                    fill=0.0, base=base, pattern=[[1, n]],
                                    channel_multiplier=-1)

        # Per-slot ALiBi attention window, in 128-tiles. Core heads are
        # {hg, hg+4, hg+8, hg+12}; slot j's smallest slope bounds the dropped
        # weight mass below e^-35 of the kept mass.
        WTS = [2, 6, 16, 16]

        def attention(h):
            wt = WTS[h]
            qa, ka = qT_a[h], kT_a[h]

            def vsl(kt):
                return v_sb[kt][:, h * (D + 1):(h + 1) * (D + 1)]

            def finish_chunk(tq, psy):
                i0 = tq * CS
                dstg = mp.tile([D + 1, CS], f32, name="dstg", tag="dstg")
                nc.vector.tensor_copy(dstg[D:D + 1, :], psy[D:D + 1, :])
                nc.sync.dma_start(denom_sb[h][tq:tq + 1, :], dstg[D:D + 1, :])
                if h % 2 == 0:
                    nc.vector.tensor_copy(r(yT_sb[h // 2][0:D, i0:i0 + CS]),
                                          psy[0:D, :])
                else:
                    ystg = mp.tile([D, CS], f32, name="ystg", tag="ystg")
                    nc.vector.tensor_copy(r(ystg), psy[0:D, :])
                    nc.sync.dma_start(r(yT_sb[h // 2][D:2 * D, i0:i0 + CS]),
                                      r(ystg))

            # (psy column offset, matmul N) for diagonal tile d; d=3 computes
            # an extra masked 128 cols to stay at the fp32r full-rate N>=256.
            DIAG = [(0, CS), (P, CS - P), (256, 256), (256, 256)]

            # Tq chunks in pairs: each ka/v stationary operand is loaded once
            # per up-to-two matmuls (LDWEIGHTS otherwise dominates).
            for t in range(NQ // 2):
                tq0, tq1 = 2 * t, 2 * t + 1
                lo0, lo1 = max(0, 4 * tq0 - wt), max(0, 4 * tq1 - wt)
                psy0 = ps_y.tile([D + 1, CS], f32, name="psy0", tag="y")
                psy1 = ps_y.tile([D + 1, CS], f32, name="psy1", tag="y")
                for kt in range(lo0, 4 * tq1 + 4):
                    steps = []
                    for tq, psy, lo in ((tq0, psy0, lo0), (tq1, psy1, lo1)):
                        d = kt - 4 * tq
                        if kt < lo or d > 3:
                            continue
                        o, n = (0, CS) if d < 0 else DIAG[d]
                        steps.append((psy, tq * CS, o, n,
                                      None if d < 0 else o - P * d,
                                      kt == lo, d == 3, tq))
                    kasl = ka[:, kt * P:(kt + 1) * P]
                    pb = ps_s.tile([P, 2 * CS], f32, name="pb", tag="sbig")
                    # a matmul output must stay inside one 512-f32 PSUM bank:
                    # put the full-width step at col 0 and any second step at
                    # col CS (bank 1).
                    steps.sort(key=lambda st: -st[3])
                    cols = [0, CS][:len(steps)]
                    col = (CS + steps[1][3]) if len(steps) == 2 else steps[0][3]
                    for c0, (_, i0_, o, n, _, _, _, _) in zip(cols, steps):
                        mm(pb[:, c0:c0 + n], kasl,
                           qa[:, i0_ + o:i0_ + o + n], True, True)
                    eb = ep.tile([P, 2 * CS], f32, name="eb", tag="e")
                    nc.scalar.activation(r(eb[:, 0:col]), pb[:, 0:col], EXP)
                    for c0, (_, _, o, n, sb, _, _, _) in zip(cols, steps):
                        if sb is not None:
                            sel(eb[:, c0:c0 + n], n, sb)
                    vv = vsl(kt)
                    for c0, (psy, _, o, n, _, st, sp, _) in zip(cols, steps):
                        mm(psy[:, o:o + n], vv, eb[:, c0:c0 + n],
                           start=st, stop=sp)
                    for c0, (psy, _, _, _, _, _, sp, tq) in zip(cols, steps):
                        if sp and tq == tq0:
                            finish_chunk(tq0, psy0)
                finish_chunk(tq1, psy1)

        def normalize(h):
            nc.vector.reciprocal_approx_fast(out=denom_sb[h], in_=denom_sb[h])
            for tq in range(NQ):
                rtmp = mp.tile([1, CS], f32, name="rtmp", tag="rtmp", bufs=1)
                nc.sync.dma_start(rtmp, denom_sb[h][tq:tq + 1, :])
                rb = mp.tile([P, CS], f32, name="rb", tag="rb")
                nc.gpsimd.partition_broadcast(rb, rtmp)
                r0 = (h % 2) * D
                ys = yT_sb[h // 2][r0:r0 + D, tq * CS:(tq + 1) * CS]
                nc.vector.tensor_mul(r(ys), ys, rb[r0:r0 + D, :])

        # ---- qT/kT per head pair, v between, attention + normalize per head --
        qT_a, kT_a = {}, {}

        def qk_pair(m):
            for j in range(2):
                h = 2 * m + j
                qT_a[h] = kqp.tile([D + 2, T], f32, name=f"qTa{h}", tag="qTa")
                kT_a[h] = kqp.tile([D + 2, T], f32, name=f"kTa{h}", tag="kTa")
                nc.sync.dma_start(r(qT_a[h][D:D + 2, :]), r(qaug[h]))
                nc.sync.dma_start(r(kT_a[h][D:D + 2, :]), r(kaug))
            for tqp in range(NQ // 2):
                tq0, tq1 = 2 * tqp, 2 * tqp + 1
                for nm, dst in (("q", qT_a), ("k", kT_a)):
                    ps0 = ps_mm.tile([P, CS], f32, name=f"ps{nm}0", tag="mm")
                    ps1 = ps_mm.tile([P, CS], f32, name=f"ps{nm}1", tag="mm")
                    for ci in range(CI):
                        w = w_sb[nm, ci][:, m * P:(m + 1) * P]
                        mm(ps0, w, x_sb[ci][:, tq0 * CS:(tq0 + 1) * CS],
                           start=ci == 0, stop=ci == CI - 1)
                        mm(ps1, w, x_sb[ci][:, tq1 * CS:(tq1 + 1) * CS],
                           start=ci == 0, stop=ci == CI - 1)
                    for tq, ps in ((tq0, ps0), (tq1, ps1)):
                        # even head: direct copy; odd head: stage + DMA shift
                        nc.vector.tensor_copy(
                            r(dst[2 * m][0:D, tq * CS:(tq + 1) * CS]), ps[0:D, :])
                        stg = mp.tile([P, CS], f32, name=f"stg{nm}", tag="stg")
                        nc.vector.tensor_copy(r(stg[D:P, :]), ps[D:P, :])
                        nc.sync.dma_start(
                            r(dst[2 * m + 1][0:D, tq * CS:(tq + 1) * CS]),
                            r(stg[D:P, :]))

        qk_pair(0)

        # ---- v = x @ Wv^T (wv/wp loads deferred behind the qk-critical DMAs)
        for ci in range(CI):
            t = wpool.tile([P, HL * D], f32, name=f"wv{ci}", tag=f"wv{ci}")
            nc.sync.dma_start(r(t), r(wvT[ci * P:(ci + 1) * P, :]))
            w_sb["v", ci] = t
        wp_sb = []
        for i in range(2):
            t = wpool.tile([P, C], f32, name=f"wp{i}", tag=f"wp{i}")
            nc.sync.dma_start(r(t), r(wpT[i * P:(i + 1) * P, :]))
            wp_sb.append(t)
        for tt in range(TT):
            psv = ps_mm.tile([P, HL * D], f32, name="psv", tag="mm")
            for ci in range(CI):
                mm(psv, x_sb[ci][:, tt * P:(tt + 1) * P], w_sb["v", ci],
                   start=ci == 0, stop=ci == CI - 1)
            vt = vp.tile([P, HL * (D + 1)], f32, name=f"v{tt}", tag=f"v{tt}")
            v3 = vt.rearrange("p (h e) -> p h e", h=HL)
            nc.sync.dma_start(r(v3[:, :, D:D + 1]), r(vones.unsqueeze(2)))
            nc.vector.tensor_copy(r(v3[:, :, 0:D]),
                                  psv.rearrange("p (h d) -> p h d", h=HL))
            v_sb.append(vt)

        attention(0)
        normalize(0)
        attention(1)
        normalize(1)
        qk_pair(1)
        attention(2)
        normalize(2)
        attention(3)
        normalize(3)

        # ---- partial projection --------------------------------------------
        for tt in range(TT):
            pp0 = ps_mm.tile([P, CS], f32, name="pp0", tag="mm")
            pp1 = ps_mm.tile([P, CS], f32, name="pp1", tag="mm")
            for kc in range(2):
                lh = yT_sb[kc][:, tt * P:(tt + 1) * P]
                mm(pp0, lh, wp_sb[kc][:, 0:CS], start=kc == 0, stop=kc == 1)
                mm(pp1, lh, wp_sb[kc][:, CS:2 * CS], start=kc == 0, stop=kc == 1)
            for nh, pp in ((0, pp0), (1, pp1)):
                ot = op_pool.tile([P, CS], f32, name="ot", tag="o")
                nc.vector.tensor_copy(ot, pp)
                nc.sync.dma_start(
                    outp[tt * P:(tt + 1) * P, nh * CS:(nh + 1) * CS], ot)

    nc.compile()
    _BUILT["nc"] = nc
    return nc


def _prep_inputs(x, w_attn, w_proj):
    """Shard + lay out the full inputs for the 8 cores."""
    x = np.asarray(x, dtype=np.float32)
    w_attn = np.asarray(w_attn, dtype=np.float32)
    w_proj = np.asarray(w_proj, dtype=np.float32)

    slopes = _alibi_slopes(H)
    iota = np.arange(T, dtype=np.float32)
    kaug = np.stack([iota, np.ones(T, np.float32)])  # (2, T)
    xTs = [np.ascontiguousarray(x[b].T) for b in range(B)]

    in_maps = []
    for c in range(N_CORES):
        b, hg = divmod(c, 4)
        heads = [hg, hg + 4, hg + 8, hg + 12]  # slot j gets window WTS[j]
        rows = np.concatenate([np.arange(h * D, (h + 1) * D) for h in heads])
        qaug = np.empty((HL, 2, T), np.float32)
        for j, h in enumerate(heads):
            s = slopes[h]
            qaug[j, 0, :] = s
            qaug[j, 1, :] = -s * iota
        in_maps.append({
            "xT": xTs[b],
            "wqT": np.ascontiguousarray(w_attn[rows, :].T) * np.float32(0.125),
            "wkT": np.ascontiguousarray(w_attn[C + rows, :].T),
            "wvT": np.ascontiguousarray(w_attn[2 * C + rows, :].T),
            "wpT": np.ascontiguousarray(w_proj[:, rows].T),
            "kaug": kaug,
            "vones": np.ones((P, HL), np.float32),
            "qaug": qaug,
        })
    return in_maps


def kernel(x, w_attn, w_proj):
    from concourse import bass_utils

    nc = _build()
    in_maps = _prep_inputs(x, w_attn, w_proj)
    res = bass_utils.run_bass_kernel_spmd(nc, in_maps, core_ids=list(range(N_CORES)))
    out = np.zeros((B, T, C), dtype=np.float32)
    for c in range(N_CORES):
        out[c // 4] += res.results[c]["outp"]
    return out



# revision 10
# speedup vs baseline: 1.2387x; 1.2387x over previous
"""Causal self-attention with ALiBi for Trainium2, sharded over 8 NeuronCores.

Problem: B=2, T=2048, C=1024, H=16 heads, D=64. y = proj(softmax(qk^T/8 + alibi) v).

Sharding (per spec hint): data-parallel on B x tensor-parallel on heads.
Core c handles batch b = c // 4 and the 4 heads [4*(c%4), 4*(c%4)+4).
Each core computes its heads' attention output and a partial projection
(contracting only its 256 columns of w_proj); the host sums the 4 partials
per batch.

Host-side prep (not device work): x is pre-transposed to xT=(C,T) per batch
and cast to bf16; weights are pre-sliced/transposed/cast/packed per core so
the device kernel needs no on-chip transposes and loads each weight group
with a single wide DMA. The attention scale 1/8 is folded into wq.

Dtype strategy: all GEMMs whose operands tolerate 8-bit mantissas run in
bf16 (qkv projections, attention e@v, output projection) - bf16 matmuls use
fast weight load so the per-matmul LDWEIGHTS cost mostly disappears. Only
the qk+ALiBi matmul stays float32r (fp22): its augmented iota rows encode
s*(j-i) exactly and would be destroyed by bf16's 8-bit mantissa. Note
bf16 stationary operands must start at 4-byte-aligned offsets, hence the
per-head v stride of D+2 lanes.

Device pipeline per core (f = feature-major layout):
  1. qT/kT = W^T-slices @ xT     -> (64, T) per head, f32 (fp32r consumer).
  2. v     = x @ Wv^T            -> (T, 4*(D+2)) natural bf16, ones column
     at lane D per head (softmax denominator for free).
  3. ALiBi via 2 extra contraction rows: k-side [j; 1], q-side
     [slope; -slope*i] => s_T[j,i] = q.k/8 + slope*(j-i), K=66, fp32r.
  4. e_T = exp(s_T) on ACT (bf16 out); causal mask applied by zeroing
     e_T's upper triangle on GpSimd (affine_select) for diagonal tiles.
  5. y_aug^T = [v | 1]^T @ e_T (bf16) accumulated over Tk tiles.
  6. normalize via batched reciprocal + partition-broadcast multiply.
  7. partial out = y^T.T @ wp^T-slice (bf16), split into a kc=0 partial
     (stashed in SBUF) and a kc=1 pass that adds and DMAs out; host
     accumulates partials in f32.

Scheduling: the attention inner loop is software-pipelined (the e@v matmul
for slab k is emitted after slab k+1's qk matmul + exp, so the FIFO PE
queue never head-of-line blocks on ACT), and the projection-phase work is
interleaved into the ACT-bound attention streams via generators:
  qk_pair(0) -> attention(0) || v-loop -> attention(1) || qk_pair(1)
  -> attention(2..3) || proj kc=0 -> proj kc=1 + output DMA.
This keeps the PE dense (HAM clock gate stays warm) and fills its slack
while ACT streams exp. PSUM evacuations in the DMA-bound front section run
on the otherwise-idle ACT engine; attention-phase evacuations stay on DVE.

Per-slot ALiBi windows WTS are sized so the dropped weight mass is
< e^-18 of the kept mass (rel-err budget is 2e-2; measured total error
~6e-3, dominated by bf16 rounding, not windowing).

DVE/ACT engines are partition-locked (operands must share the start
partition), so moving a head's 64 rows from psum partitions 64:128 down to
0:64 goes through a small SBUF->SBUF DMA (staging tile) instead.
"""

import math

import numpy as np

B, T, C = 2, 2048, 1024
H, D = 16, 64
HL = 4          # heads per core
N_CORES = 8
P = 128         # partitions
CS = 512        # Tq chunk (matmul moving dim)
CI = C // P     # 8 contraction chunks
TT = T // P     # 16 T tiles
NQ = T // CS    # 4 Tq chunks
HLD = HL * D    # 256
VL = D + 2      # per-head v lane stride (even => 4B-aligned bf16 slices)

_BUILT = {}


def _alibi_slopes(n_heads):
    start = 2.0 ** (-(2.0 ** (-(math.log2(n_heads) - 3))))
    return np.array([start * start**i for i in range(n_heads)], dtype=np.float32)


def _drive(main, filler, ratio):
    """Run generator `main` to completion, advancing `filler` ~ratio steps
    per main step (interleaves instruction emission across phases)."""
    acc = 0.0
    done = filler is None
    for _ in main:
        if done:
            continue
        acc += ratio
        while acc >= 1.0:
            acc -= 1.0
            if next(filler, _SENT) is _SENT:
                done = True
                break
    if not done:
        for _ in filler:
            pass


_SENT = object()


def _build():
    """Build + compile the (single, SPMD) Bass module. Cached per process."""
    if "nc" in _BUILT:
        return _BUILT["nc"]

    from contextlib import ExitStack

    import concourse.bacc as bacc
    import concourse.mybir as mybir
    import concourse.tile as tile

    f32 = mybir.dt.float32
    f32r = mybir.dt.float32r
    bf16 = mybir.dt.bfloat16
    EXP = mybir.ActivationFunctionType.Exp
    GE = mybir.AluOpType.is_ge

    nc = bacc.Bacc("TRN2", target_bir_lowering=False)

    xT = nc.dram_tensor("xT", [C, T], bf16, kind="ExternalInput").ap()
    # wqk: [128, 2*CI*HLD] = wq|wk, ci-major blocks; wvp: wv | wp(2 rows of C)
    wqk = nc.dram_tensor("wqk", [P, 2 * CI * HLD], bf16, kind="ExternalInput").ap()
    wvp = nc.dram_tensor("wvp", [P, CI * HLD + 2 * C], bf16,
                         kind="ExternalInput").ap()
    kaug = nc.dram_tensor("kaug", [2, T], f32, kind="ExternalInput").ap()
    qaug = nc.dram_tensor("qaug", [HL, 2, T], f32, kind="ExternalInput").ap()
    outp = nc.dram_tensor("outp", [T, C], bf16, kind="ExternalOutput").ap()

    def mm(out, lhsT, rhs, start, stop):
        # fp32r matmul (qk with ALiBi aug rows)
        nc.tensor.matmul(out, lhsT.bitcast(f32r), rhs.bitcast(f32r),
                         start=start, stop=stop)

    def mmb(out, lhsT, rhs, start, stop):
        # bf16 matmul
        nc.tensor.matmul(out, lhsT, rhs, start=start, stop=stop)

    def r(ap):
        # walrus requires every writer of an fp32r-matmul operand to declare
        # fp32r output; the PE truncates to fp22 on read either way.
        return ap.bitcast(f32r)

    with tile.TileContext(nc) as tc, ExitStack() as ctx:
        xp = ctx.enter_context(tc.tile_pool(name="xp", bufs=1))
        wpool = ctx.enter_context(tc.tile_pool(name="wpool", bufs=1))
        vp = ctx.enter_context(tc.tile_pool(name="vp", bufs=1))
        kqp = ctx.enter_context(tc.tile_pool(name="kqp", bufs=3))
        ep = ctx.enter_context(tc.tile_pool(name="ep", bufs=4))
        yp = ctx.enter_context(tc.tile_pool(name="yp", bufs=1))
        ob = ctx.enter_context(tc.tile_pool(name="ob", bufs=1))
        mp = ctx.enter_context(tc.tile_pool(name="mp", bufs=2))
        op_pool = ctx.enter_context(tc.tile_pool(name="op", bufs=2))
        ps_mm = ctx.enter_context(tc.tile_pool(name="ps_mm", bufs=2, space="PSUM"))
        ps_s = ctx.enter_context(tc.tile_pool(name="ps_s", bufs=2, space="PSUM"))
        ps_y = ctx.enter_context(tc.tile_pool(name="ps_y", bufs=2, space="PSUM"))

        # ---- resident loads: x per chunk (compute starts on chunk 0), the
        #      packed weight groups in one wide DMA each.
        x_sb = []
        for ci in range(CI):
            t = xp.tile([P, T], bf16, name=f"x{ci}", tag=f"x{ci}")
            nc.sync.dma_start(t, xT[ci * P:(ci + 1) * P, :])
            x_sb.append(t)
        wqk_sb = wpool.tile([P, 2 * CI * HLD], bf16, name="wqk_sb", tag="wqk")
        nc.sync.dma_start(wqk_sb, wqk)
        wvp_sb = wpool.tile([P, CI * HLD + 2 * C], bf16, name="wvp_sb", tag="wvp")
        nc.sync.dma_start(wvp_sb, wvp)

        def w_sl(nm, ci):
            base = {"q": 0, "k": CI * HLD}[nm] + ci * HLD
            return wqk_sb[:, base:base + HLD]

        def wv_sl(ci):
            return wvp_sb[:, ci * HLD:(ci + 1) * HLD]

        def wp_sl(kc):
            base = CI * HLD + kc * C
            return wvp_sb[:, base:base + C]

        yT_sb = [yp.tile([P, T], bf16, name=f"yT{i}", tag=f"yT{i}") for i in range(2)]
        obuf = [ob.tile([P, 2 * CS], bf16, name=f"ob{tt}", tag=f"ob{tt}")
                for tt in range(TT)]
        denom_sb = [mp.tile([NQ, CS], f32, name=f"denom{i}", tag=f"denom{i}",
                            bufs=1) for i in range(HL)]

        v_sb = []

        def sel(ap, n, base):
            # zero the causally-masked region: keep where free - part + base >= 0
            nc.gpsimd.affine_select(out=ap, in_=ap, compare_op=GE,
                                    fill=0.0, base=base, pattern=[[1, n]],
                                    channel_multiplier=-1)

        # Per-slot ALiBi attention window, in 128-tiles. Core heads are
        # {hg, hg+4, hg+8, hg+12}; slot j's smallest slope (head 4j+3 across
        # cores) bounds the dropped weight mass below e^-18 of the kept mass.
        WTS = [1, 3, 9, 16]

        def attention_gen(h):
            wt = WTS[h]
            qa, ka = qT_a[h], kT_a[h]

            def vsl(kt):
                return v_sb[kt][:, h * VL:h * VL + D + 1]

            def finish_chunk(tq, psy):
                i0 = tq * CS
                dstg = mp.tile([D + 1, CS], f32, name="dstg", tag="dstg")
                nc.vector.tensor_copy(dstg[D:D + 1, :], psy[D:D + 1, :])
                nc.sync.dma_start(denom_sb[h][tq:tq + 1, :], dstg[D:D + 1, :])
                if h % 2 == 0:
                    nc.vector.tensor_copy(yT_sb[h // 2][0:D, i0:i0 + CS],
                                          psy[0:D, :])
                else:
                    ystg = mp.tile([D, CS], bf16, name="ystg", tag="ystg")
                    nc.vector.tensor_copy(ystg, psy[0:D, :])
                    nc.sync.dma_start(yT_sb[h // 2][D:2 * D, i0:i0 + CS], ystg)

            # (psy column offset, matmul N) for diagonal tile d; d=3 computes
            # an extra masked 128 cols to stay at the fp32r full-rate N>=256.
            DIAG = [(0, CS), (P, CS - P), (256, 256), (256, 256)]

            # Software pipeline: the e@v matmuls for a slab are emitted after
            # the NEXT slab's qk matmuls + exp, so the PE never head-of-line
            # blocks on ACT.
            def emit_av(p):
                cols_, steps_, eb_, kt_ = p
                vv_ = vsl(kt_)
                for c0, (psy, _, o, n, _, st, sp, _) in zip(cols_, steps_):
                    mmb(psy[:, o:o + n], vv_, eb_[:, c0:c0 + n],
                        start=st, stop=sp)
                for c0, (psy, _, _, _, _, _, sp, tq) in zip(cols_, steps_):
                    if sp:
                        pend_fin.append((tq, psy))

            for t in range(NQ // 2):
                tq0, tq1 = 2 * t, 2 * t + 1
                lo0, lo1 = max(0, 4 * tq0 - wt), max(0, 4 * tq1 - wt)
                psy0 = ps_y.tile([D + 1, CS], f32, name="psy0", tag="y")
                psy1 = ps_y.tile([D + 1, CS], f32, name="psy1", tag="y")
                pend = None
                pend_fin = []
                for kt in range(lo0, 4 * tq1 + 4):
                    steps = []
                    for tq, psy, lo in ((tq0, psy0, lo0), (tq1, psy1, lo1)):
                        d = kt - 4 * tq
                        if kt < lo or d > 3:
                            continue
                        o, n = (0, CS) if d < 0 else DIAG[d]
                        steps.append((psy, tq * CS, o, n,
                                      None if d < 0 else o - P * d,
                                      kt == lo, d == 3, tq))
                    kasl = ka[:, kt * P:(kt + 1) * P]
                    pb = ps_s.tile([P, 2 * CS], f32, name="pb", tag="sbig")
                    # a matmul output must stay inside one 512-f32 PSUM bank:
                    # put the full-width step at col 0 and any second step at
                    # col CS (bank 1).
                    steps.sort(key=lambda st: -st[3])
                    cols = [0, CS][:len(steps)]
                    col = (CS + steps[1][3]) if len(steps) == 2 else steps[0][3]
                    for c0, (_, i0_, o, n, _, _, _, _) in zip(cols, steps):
                        mm(pb[:, c0:c0 + n], kasl,
                           qa[:, i0_ + o:i0_ + o + n], True, True)
                    eb = ep.tile([P, 2 * CS], bf16, name="eb", tag="e")
                    nc.scalar.activation(eb[:, 0:col], pb[:, 0:col], EXP)
                    for c0, (_, _, o, n, sb, _, _, _) in zip(cols, steps):
                        if sb is not None:
                            sel(eb[:, c0:c0 + n], n, sb)
                    if pend is not None:
                        emit_av(pend)
                        while pend_fin:
                            tq, psy = pend_fin.pop(0)
                            finish_chunk(tq, psy)
                    pend = (cols, steps, eb, kt)
                    yield
                emit_av(pend)
                while pend_fin:
                    tq, psy = pend_fin.pop(0)
                    finish_chunk(tq, psy)

        def normalize(h):
            nc.vector.reciprocal_approx_fast(out=denom_sb[h], in_=denom_sb[h])
            denb = mp.tile([NQ, CS], bf16, name="denb", tag="denb")
            nc.vector.tensor_copy(denb, denom_sb[h])
            for tq in range(NQ):
                rtmp = mp.tile([1, CS], bf16, name="rtmp", tag="rtmp", bufs=1)
                nc.sync.dma_start(rtmp, denb[tq:tq + 1, :])
                rb = mp.tile([P, CS], bf16, name="rb", tag="rb")
                nc.gpsimd.partition_broadcast(rb, rtmp)
                r0 = (h % 2) * D
                ys = yT_sb[h // 2][r0:r0 + D, tq * CS:(tq + 1) * CS]
                nc.vector.tensor_mul(ys, ys, rb[r0:r0 + D, :])

        # ---- qT/kT per head pair -------------------------------------------
        qT_a, kT_a = {}, {}

        def qk_pair_gen(m, on_act):
            evac = nc.scalar.copy if on_act else (
                lambda out, in_: nc.vector.tensor_copy(out, in_))
            for j in range(2):
                h = 2 * m + j
                qT_a[h] = kqp.tile([D + 2, T], f32, name=f"qTa{h}", tag="qTa")
                kT_a[h] = kqp.tile([D + 2, T], f32, name=f"kTa{h}", tag="kTa")
                nc.sync.dma_start(r(qT_a[h][D:D + 2, :]), r(qaug[h]))
                nc.sync.dma_start(r(kT_a[h][D:D + 2, :]), r(kaug))
            for tqp in range(NQ // 2):
                tq0, tq1 = 2 * tqp, 2 * tqp + 1
                for nm, dst in (("q", qT_a), ("k", kT_a)):
                    ps0 = ps_mm.tile([P, CS], f32, name=f"ps{nm}0", tag="mm")
                    ps1 = ps_mm.tile([P, CS], f32, name=f"ps{nm}1", tag="mm")
                    for ci in range(CI):
                        w = w_sl(nm, ci)[:, m * P:(m + 1) * P]
                        mmb(ps0, w, x_sb[ci][:, tq0 * CS:(tq0 + 1) * CS],
                            start=ci == 0, stop=ci == CI - 1)
                        mmb(ps1, w, x_sb[ci][:, tq1 * CS:(tq1 + 1) * CS],
                            start=ci == 0, stop=ci == CI - 1)
                        yield
                    for tq, ps in ((tq0, ps0), (tq1, ps1)):
                        # even head: direct copy; odd head: stage + DMA shift
                        evac(r(dst[2 * m][0:D, tq * CS:(tq + 1) * CS]),
                             ps[0:D, :])
                        stg = mp.tile([P, CS], f32, name=f"stg{nm}", tag="stg")
                        evac(r(stg[D:P, :]), ps[D:P, :])
                        nc.sync.dma_start(
                            r(dst[2 * m + 1][0:D, tq * CS:(tq + 1) * CS]),
                            r(stg[D:P, :]))

        # ---- v = x @ Wv^T ---------------------------------------------------
        def v_gen():
            for tt in range(TT):
                psv = ps_mm.tile([P, HLD], f32, name="psv", tag="mm")
                for ci in range(CI):
                    mmb(psv, x_sb[ci][:, tt * P:(tt + 1) * P], wv_sl(ci),
                        start=ci == 0, stop=ci == CI - 1)
                vt = vp.tile([P, HL * VL], bf16, name=f"v{tt}", tag=f"v{tt}")
                v3 = vt.rearrange("p (h e) -> p h e", h=HL)
                nc.vector.memset(v3[:, :, D:D + 2], 1.0)
                nc.vector.tensor_copy(v3[:, :, 0:D],
                                      psv.rearrange("p (h d) -> p h d", h=HL))
                v_sb.append(vt)
                yield

        # ---- partial projection: kc=0 stash, kc=1 add + DMA out -------------
        def proj_a_gen():
            for tt in range(TT):
                pp0 = ps_mm.tile([P, CS], f32, name="pp0", tag="mm")
                pp1 = ps_mm.tile([P, CS], f32, name="pp1", tag="mm")
                lh = yT_sb[0][:, tt * P:(tt + 1) * P]
                mmb(pp0, lh, wp_sl(0)[:, 0:CS], start=True, stop=True)
                mmb(pp1, lh, wp_sl(0)[:, CS:2 * CS], start=True, stop=True)
                yield
                nc.vector.tensor_copy(obuf[tt][:, 0:CS], pp0)
                nc.vector.tensor_copy(obuf[tt][:, CS:2 * CS], pp1)
                yield

        def proj_b():
            for tt in range(TT):
                pp0 = ps_mm.tile([P, CS], f32, name="pp0b", tag="mm")
                pp1 = ps_mm.tile([P, CS], f32, name="pp1b", tag="mm")
                lh = yT_sb[1][:, tt * P:(tt + 1) * P]
                mmb(pp0, lh, wp_sl(1)[:, 0:CS], start=True, stop=True)
                mmb(pp1, lh, wp_sl(1)[:, CS:2 * CS], start=True, stop=True)
                ot = op_pool.tile([P, 2 * CS], bf16, name="ot", tag="o")
                nc.vector.tensor_add(ot[:, 0:CS], pp0, obuf[tt][:, 0:CS])
                nc.vector.tensor_add(ot[:, CS:2 * CS], pp1, obuf[tt][:, CS:2 * CS])
                nc.sync.dma_start(outp[tt * P:(tt + 1) * P, :], ot)

        # ---- schedule -------------------------------------------------------
        _drive(qk_pair_gen(0, True), None, 0)
        _drive(attention_gen(0), v_gen(), 1.0)
        normalize(0)
        _drive(attention_gen(1), qk_pair_gen(1, False), 1.6)
        normalize(1)
        pa = proj_a_gen()
        _drive(attention_gen(2), pa, 0.7)
        normalize(2)
        _drive(attention_gen(3), pa, 0.7)
        normalize(3)
        proj_b()

    nc.compile()
    _BUILT["nc"] = nc
    return nc


def _prep_inputs(x, w_attn, w_proj):
    """Shard + lay out the full inputs for the 8 cores."""
    import ml_dtypes

    bf16 = ml_dtypes.bfloat16
    x = np.asarray(x, dtype=np.float32)
    w_attn = np.asarray(w_attn, dtype=np.float32)
    w_proj = np.asarray(w_proj, dtype=np.float32)

    slopes = _alibi_slopes(H)
    iota = np.arange(T, dtype=np.float32)
    kaug = np.stack([iota, np.ones(T, np.float32)])  # (2, T)
    xTs = [np.ascontiguousarray(x[b].T).astype(bf16) for b in range(B)]

    def ci_major(w):  # (C, HLD) -> (P, CI*HLD)
        return np.ascontiguousarray(
            w.reshape(CI, P, HLD).transpose(1, 0, 2).reshape(P, CI * HLD))

    in_maps = []
    for c in range(N_CORES):
        b, hg = divmod(c, 4)
        heads = [hg, hg + 4, hg + 8, hg + 12]  # slot j gets window WTS[j]
        rows = np.concatenate([np.arange(h * D, (h + 1) * D) for h in heads])
        qaug = np.empty((HL, 2, T), np.float32)
        for j, h in enumerate(heads):
            s = slopes[h]
            qaug[j, 0, :] = s
            qaug[j, 1, :] = -s * iota
        wq = np.ascontiguousarray(w_attn[rows, :].T) * np.float32(0.125)
        wk = np.ascontiguousarray(w_attn[C + rows, :].T)
        wv = np.ascontiguousarray(w_attn[2 * C + rows, :].T)
        wp = np.ascontiguousarray(w_proj[:, rows].T)  # (HLD, C)
        wqk = np.concatenate([ci_major(wq), ci_major(wk)], axis=1).astype(bf16)
        wvp = np.concatenate(
            [ci_major(wv),
             wp.reshape(2, P, C).transpose(1, 0, 2).reshape(P, 2 * C)],
            axis=1).astype(bf16)
        in_maps.append({
            "xT": xTs[b],
            "wqk": wqk,
            "wvp": wvp,
            "kaug": kaug,
            "qaug": qaug,
        })
    return in_maps


def kernel(x, w_attn, w_proj):
    from concourse import bass_utils

    nc = _build()
    in_maps = _prep_inputs(x, w_attn, w_proj)
    res = bass_utils.run_bass_kernel_spmd(nc, in_maps, core_ids=list(range(N_CORES)))
    out = np.zeros((B, T, C), dtype=np.float32)
    for c in range(N_CORES):
        out[c // 4] += np.asarray(res.results[c]["outp"], dtype=np.float32)
    return out


# revision 12
# speedup vs baseline: 1.2565x; 1.0144x over previous
"""Causal self-attention with ALiBi for Trainium2, sharded over 8 NeuronCores.

Problem: B=2, T=2048, C=1024, H=16 heads, D=64. y = proj(softmax(qk^T/8 + alibi) v).

Sharding (per spec hint): data-parallel on B x tensor-parallel on heads.
Core c handles batch b = c // 4 and the 4 heads [4*(c%4), 4*(c%4)+4).
Each core computes its heads' attention output and a partial projection
(contracting only its 256 columns of w_proj); the host sums the 4 partials
per batch.

Host-side prep (not device work): x is pre-transposed to xT=(C,T) per batch
and cast to bf16; weights are pre-sliced/transposed/cast/packed per core so
the device kernel needs no on-chip transposes and loads each weight group
with a single wide DMA. The attention scale 1/8 is folded into wq.

Dtype strategy: all GEMMs whose operands tolerate 8-bit mantissas run in
bf16 (qkv projections, attention e@v, output projection) - bf16 matmuls use
fast weight load so the per-matmul LDWEIGHTS cost mostly disappears. Only
the qk+ALiBi matmul stays float32r (fp22): its augmented iota rows encode
s*(j-i) exactly and would be destroyed by bf16's 8-bit mantissa. Note
bf16 stationary operands must start at 4-byte-aligned offsets, hence the
per-head v stride of D+2 lanes.

Device pipeline per core (f = feature-major layout):
  1. qT/kT = W^T-slices @ xT     -> (64, T) per head, f32 (fp32r consumer).
  2. v     = x @ Wv^T            -> (T, 4*(D+2)) natural bf16, ones column
     at lane D per head (softmax denominator for free).
  3. ALiBi via 2 extra contraction rows: k-side [j; 1], q-side
     [slope; -slope*i] => s_T[j,i] = q.k/8 + slope*(j-i), K=66, fp32r.
  4. e_T = exp(s_T) on ACT (bf16 out); causal mask applied by zeroing
     e_T's upper triangle on GpSimd (affine_select) for diagonal tiles.
  5. y_aug^T = [v | 1]^T @ e_T (bf16) accumulated over Tk tiles.
  6. normalize via batched reciprocal + partition-broadcast multiply.
  7. partial out = y^T.T @ wp^T-slice (bf16), split into a kc=0 partial
     (stashed in SBUF) and a kc=1 pass that adds and DMAs out; host
     accumulates partials in f32.

Scheduling: the attention inner loop is software-pipelined (the e@v matmul
for slab k is emitted after slab k+1's qk matmul + exp, so the FIFO PE
queue never head-of-line blocks on ACT), and the projection-phase work is
interleaved into the ACT-bound attention streams via generators:
  qk_pair(0) -> attention(0) || v-loop -> attention(1) || qk_pair(1)
  -> attention(2..3) || proj kc=0 -> proj kc=1 + output DMA.
This keeps the PE dense (HAM clock gate stays warm) and fills its slack
while ACT streams exp. PSUM evacuations in the DMA-bound front section run
on the otherwise-idle ACT engine; attention-phase evacuations stay on DVE.

Per-slot ALiBi windows WTS are sized so the dropped weight mass is
< e^-18 of the kept mass (rel-err budget is 2e-2; measured total error
~6e-3, dominated by bf16 rounding, not windowing).

DVE/ACT engines are partition-locked (operands must share the start
partition), so moving a head's 64 rows from psum partitions 64:128 down to
0:64 goes through a small SBUF->SBUF DMA (staging tile) instead.
"""

import math

import numpy as np

B, T, C = 2, 2048, 1024
H, D = 16, 64
HL = 4          # heads per core
N_CORES = 8
P = 128         # partitions
CS = 512        # Tq chunk (matmul moving dim)
CI = C // P     # 8 contraction chunks
TT = T // P     # 16 T tiles
NQ = T // CS    # 4 Tq chunks
HLD = HL * D    # 256
VL = D + 2      # per-head v lane stride (even => 4B-aligned bf16 slices)

_BUILT = {}


def _alibi_slopes(n_heads):
    start = 2.0 ** (-(2.0 ** (-(math.log2(n_heads) - 3))))
    return np.array([start * start**i for i in range(n_heads)], dtype=np.float32)


def _drive(main, filler, ratio):
    """Run generator `main` to completion, advancing `filler` ~ratio steps
    per main step (interleaves instruction emission across phases)."""
    acc = 0.0
    done = filler is None
    for _ in main:
        if done:
            continue
        acc += ratio
        while acc >= 1.0:
            acc -= 1.0
            if next(filler, _SENT) is _SENT:
                done = True
                break
    if not done:
        for _ in filler:
            pass


_SENT = object()


def _build():
    """Build + compile the (single, SPMD) Bass module. Cached per process."""
    if "nc" in _BUILT:
        return _BUILT["nc"]

    from contextlib import ExitStack

    import concourse.bacc as bacc
    import concourse.mybir as mybir
    import concourse.tile as tile

    f32 = mybir.dt.float32
    f32r = mybir.dt.float32r
    bf16 = mybir.dt.bfloat16
    EXP = mybir.ActivationFunctionType.Exp
    GE = mybir.AluOpType.is_ge

    nc = bacc.Bacc("TRN2", target_bir_lowering=False)

    xT = nc.dram_tensor("xT", [C, T], bf16, kind="ExternalInput").ap()
    # wqk: [128, 2*CI*HLD] = wq|wk, ci-major blocks; wvp: wv | wp(2 rows of C)
    wqk = nc.dram_tensor("wqk", [P, 2 * CI * HLD], bf16, kind="ExternalInput").ap()
    wvp = nc.dram_tensor("wvp", [P, CI * HLD + 2 * C], bf16,
                         kind="ExternalInput").ap()
    kaug = nc.dram_tensor("kaug", [2, T], f32, kind="ExternalInput").ap()
    qaug = nc.dram_tensor("qaug", [HL, 2, T], f32, kind="ExternalInput").ap()
    outp = nc.dram_tensor("outp", [T, C], bf16, kind="ExternalOutput").ap()

    def mm(out, lhsT, rhs, start, stop):
        # fp32r matmul (qk with ALiBi aug rows)
        nc.tensor.matmul(out, lhsT.bitcast(f32r), rhs.bitcast(f32r),
                         start=start, stop=stop)

    def mmb(out, lhsT, rhs, start, stop):
        # bf16 matmul
        nc.tensor.matmul(out, lhsT, rhs, start=start, stop=stop)

    def r(ap):
        # walrus requires every writer of an fp32r-matmul operand to declare
        # fp32r output; the PE truncates to fp22 on read either way.
        return ap.bitcast(f32r)

    with tile.TileContext(nc) as tc, ExitStack() as ctx:
        xp = ctx.enter_context(tc.tile_pool(name="xp", bufs=1))
        wpool = ctx.enter_context(tc.tile_pool(name="wpool", bufs=1))
        vp = ctx.enter_context(tc.tile_pool(name="vp", bufs=1))
        kqp = ctx.enter_context(tc.tile_pool(name="kqp", bufs=3))
        ep = ctx.enter_context(tc.tile_pool(name="ep", bufs=4))
        yp = ctx.enter_context(tc.tile_pool(name="yp", bufs=1))
        ob = ctx.enter_context(tc.tile_pool(name="ob", bufs=1))
        mp = ctx.enter_context(tc.tile_pool(name="mp", bufs=2))
        op_pool = ctx.enter_context(tc.tile_pool(name="op", bufs=2))
        ps_mm = ctx.enter_context(tc.tile_pool(name="ps_mm", bufs=2, space="PSUM"))
        ps_s = ctx.enter_context(tc.tile_pool(name="ps_s", bufs=2, space="PSUM"))
        ps_y = ctx.enter_context(tc.tile_pool(name="ps_y", bufs=2, space="PSUM"))

        # ---- resident loads: x per chunk (compute starts on chunk 0), the
        #      packed weight groups in one wide DMA each.
        wqk_sb = wpool.tile([P, 2 * CI * HLD], bf16, name="wqk_sb", tag="wqk")
        nc.sync.dma_start(wqk_sb, wqk)
        x_sb = []
        for ci in range(CI):
            t = xp.tile([P, T], bf16, name=f"x{ci}", tag=f"x{ci}")
            nc.sync.dma_start(t, xT[ci * P:(ci + 1) * P, :])
            x_sb.append(t)
        wvp_sb = wpool.tile([P, CI * HLD + 2 * C], bf16, name="wvp_sb", tag="wvp")
        nc.sync.dma_start(wvp_sb, wvp)

        def w_sl(nm, ci):
            base = {"q": 0, "k": CI * HLD}[nm] + ci * HLD
            return wqk_sb[:, base:base + HLD]

        def wv_sl(ci):
            return wvp_sb[:, ci * HLD:(ci + 1) * HLD]

        def wp_sl(kc):
            base = CI * HLD + kc * C
            return wvp_sb[:, base:base + C]

        yT_sb = [yp.tile([P, T], bf16, name=f"yT{i}", tag=f"yT{i}") for i in range(2)]
        obuf = [ob.tile([P, 2 * CS], bf16, name=f"ob{tt}", tag=f"ob{tt}")
                for tt in range(TT)]

        v_sb = []

        def sel(ap, n, base):
            # zero the causally-masked region: keep where free - part + base >= 0
            nc.gpsimd.affine_select(out=ap, in_=ap, compare_op=GE,
                                    fill=0.0, base=base, pattern=[[1, n]],
                                    channel_multiplier=-1)

        # Per-slot ALiBi attention window, in 128-tiles. Core heads are
        # {hg, hg+4, hg+8, hg+12}; slot j's smallest slope (head 4j+3 across
        # cores) bounds the dropped weight mass below e^-18 of the kept mass.
        WTS = [1, 3, 9, 16]

        def attention_gen(h):
            wt = WTS[h]
            qa, ka = qT_a[h], kT_a[h]

            def vsl(kt):
                return v_sb[kt][:, h * VL:h * VL + D + 1]

            def finish_chunk(tq, psy):
                # normalize inline: recip of the denominator row, broadcast,
                # multiply straight out of PSUM into yT (or the odd-head stage)
                i0 = tq * CS
                dstg = mp.tile([D + 1, CS], f32, name="dstg", tag="dstg")
                nc.vector.tensor_copy(dstg[D:D + 1, :], psy[D:D + 1, :])
                rtmp = mp.tile([1, CS], f32, name="rtmp", tag="rtmp")
                nc.sync.dma_start(rtmp, dstg[D:D + 1, :])
                # recip at partition 0: reciprocal_approx_fast's constant
                # operands live on partition 0
                nc.vector.reciprocal_approx_fast(out=rtmp, in_=rtmp)
                rcb = mp.tile([1, CS], bf16, name="rcb", tag="rcb")
                nc.vector.tensor_copy(rcb, rtmp)
                rb = mp.tile([P, CS], bf16, name="rb", tag="rb")
                nc.gpsimd.partition_broadcast(rb, rcb)
                if h % 2 == 0:
                    nc.vector.tensor_mul(yT_sb[h // 2][0:D, i0:i0 + CS],
                                         psy[0:D, :], rb[0:D, :])
                else:
                    ystg = mp.tile([D, CS], bf16, name="ystg", tag="ystg")
                    nc.vector.tensor_mul(ystg, psy[0:D, :], rb[0:D, :])
                    nc.sync.dma_start(yT_sb[h // 2][D:2 * D, i0:i0 + CS], ystg)

            # (psy column offset, matmul N) for diagonal tile d; d=3 computes
            # an extra masked 128 cols to stay at the fp32r full-rate N>=256.
            DIAG = [(0, CS), (P, CS - P), (256, 256), (256, 256)]

            # Software pipeline: the e@v matmuls for a slab are emitted after
            # the NEXT slab's qk matmuls + exp, so the PE never head-of-line
            # blocks on ACT.
            def emit_av(p):
                cols_, steps_, eb_, kt_ = p
                vv_ = vsl(kt_)
                for c0, (psy, _, o, n, _, st, sp, _) in zip(cols_, steps_):
                    mmb(psy[:, o:o + n], vv_, eb_[:, c0:c0 + n],
                        start=st, stop=sp)
                for c0, (psy, _, _, _, _, _, sp, tq) in zip(cols_, steps_):
                    if sp:
                        pend_fin.append((tq, psy))

            for t in range(NQ // 2):
                tq0, tq1 = 2 * t, 2 * t + 1
                lo0, lo1 = max(0, 4 * tq0 - wt), max(0, 4 * tq1 - wt)
                psy0 = ps_y.tile([D + 1, CS], f32, name="psy0", tag="y")
                psy1 = ps_y.tile([D + 1, CS], f32, name="psy1", tag="y")
                pend = None
                pend_fin = []
                for kt in range(lo0, 4 * tq1 + 4):
                    steps = []
                    for tq, psy, lo in ((tq0, psy0, lo0), (tq1, psy1, lo1)):
                        d = kt - 4 * tq
                        if kt < lo or d > 3:
                            continue
                        o, n = (0, CS) if d < 0 else DIAG[d]
                        steps.append((psy, tq * CS, o, n,
                                      None if d < 0 else o - P * d,
                                      kt == lo, d == 3, tq))
                    kasl = ka[:, kt * P:(kt + 1) * P]
                    pb = ps_s.tile([P, 2 * CS], f32, name="pb", tag="sbig")
                    # a matmul output must stay inside one 512-f32 PSUM bank:
                    # put the full-width step at col 0 and any second step at
                    # col CS (bank 1).
                    steps.sort(key=lambda st: -st[3])
                    cols = [0, CS][:len(steps)]
                    col = (CS + steps[1][3]) if len(steps) == 2 else steps[0][3]
                    for c0, (_, i0_, o, n, _, _, _, _) in zip(cols, steps):
                        mm(pb[:, c0:c0 + n], kasl,
                           qa[:, i0_ + o:i0_ + o + n], True, True)
                    eb = ep.tile([P, 2 * CS], bf16, name="eb", tag="e")
                    nc.scalar.activation(eb[:, 0:col], pb[:, 0:col], EXP)
                    for c0, (_, _, o, n, sb, _, _, _) in zip(cols, steps):
                        if sb is not None:
                            sel(eb[:, c0:c0 + n], n, sb)
                    if pend is not None:
                        emit_av(pend)
                        while pend_fin:
                            tq, psy = pend_fin.pop(0)
                            finish_chunk(tq, psy)
                    pend = (cols, steps, eb, kt)
                    yield
                emit_av(pend)
                while pend_fin:
                    tq, psy = pend_fin.pop(0)
                    finish_chunk(tq, psy)

        # ---- qT/kT per head pair -------------------------------------------
        qT_a, kT_a = {}, {}

        def qk_pair_gen(m, on_act):
            evac = nc.scalar.copy if on_act else (
                lambda out, in_: nc.vector.tensor_copy(out, in_))
            for j in range(2):
                h = 2 * m + j
                qT_a[h] = kqp.tile([D + 2, T], f32, name=f"qTa{h}", tag="qTa")
                kT_a[h] = kqp.tile([D + 2, T], f32, name=f"kTa{h}", tag="kTa")
                nc.sync.dma_start(r(qT_a[h][D:D + 2, :]), r(qaug[h]))
                nc.sync.dma_start(r(kT_a[h][D:D + 2, :]), r(kaug))
            for tqp in range(NQ // 2):
                tq0, tq1 = 2 * tqp, 2 * tqp + 1
                for nm, dst in (("q", qT_a), ("k", kT_a)):
                    ps0 = ps_mm.tile([P, CS], f32, name=f"ps{nm}0", tag="mm")
                    ps1 = ps_mm.tile([P, CS], f32, name=f"ps{nm}1", tag="mm")
                    for ci in range(CI):
                        w = w_sl(nm, ci)[:, m * P:(m + 1) * P]
                        mmb(ps0, w, x_sb[ci][:, tq0 * CS:(tq0 + 1) * CS],
                            start=ci == 0, stop=ci == CI - 1)
                        mmb(ps1, w, x_sb[ci][:, tq1 * CS:(tq1 + 1) * CS],
                            start=ci == 0, stop=ci == CI - 1)
                        yield
                    for tq, ps in ((tq0, ps0), (tq1, ps1)):
                        # even head: direct copy; odd head: stage + DMA shift
                        evac(r(dst[2 * m][0:D, tq * CS:(tq + 1) * CS]),
                             ps[0:D, :])
                        stg = mp.tile([P, CS], f32, name=f"stg{nm}", tag="stg")
                        evac(r(stg[D:P, :]), ps[D:P, :])
                        nc.sync.dma_start(
                            r(dst[2 * m + 1][0:D, tq * CS:(tq + 1) * CS]),
                            r(stg[D:P, :]))

        # ---- v = x @ Wv^T ---------------------------------------------------
        def v_gen():
            for tt in range(TT):
                psv = ps_mm.tile([P, HLD], f32, name="psv", tag="mm")
                for ci in range(CI):
                    mmb(psv, x_sb[ci][:, tt * P:(tt + 1) * P], wv_sl(ci),
                        start=ci == 0, stop=ci == CI - 1)
                vt = vp.tile([P, HL * VL], bf16, name=f"v{tt}", tag=f"v{tt}")
                v3 = vt.rearrange("p (h e) -> p h e", h=HL)
                nc.vector.memset(v3[:, :, D:D + 2], 1.0)
                nc.vector.tensor_copy(v3[:, :, 0:D],
                                      psv.rearrange("p (h d) -> p h d", h=HL))
                v_sb.append(vt)
                yield

        # ---- partial projection: kc=0 stash, kc=1 add + DMA out -------------
        def proj_a_gen():
            for tt in range(TT):
                pp0 = ps_mm.tile([P, CS], f32, name="pp0", tag="mm")
                pp1 = ps_mm.tile([P, CS], f32, name="pp1", tag="mm")
                lh = yT_sb[0][:, tt * P:(tt + 1) * P]
                mmb(pp0, lh, wp_sl(0)[:, 0:CS], start=True, stop=True)
                mmb(pp1, lh, wp_sl(0)[:, CS:2 * CS], start=True, stop=True)
                yield
                nc.vector.tensor_copy(obuf[tt][:, 0:CS], pp0)
                nc.vector.tensor_copy(obuf[tt][:, CS:2 * CS], pp1)
                yield

        def proj_b():
            for tt in range(TT):
                pp0 = ps_mm.tile([P, CS], f32, name="pp0b", tag="mm")
                pp1 = ps_mm.tile([P, CS], f32, name="pp1b", tag="mm")
                lh = yT_sb[1][:, tt * P:(tt + 1) * P]
                mmb(pp0, lh, wp_sl(1)[:, 0:CS], start=True, stop=True)
                mmb(pp1, lh, wp_sl(1)[:, CS:2 * CS], start=True, stop=True)
                ot = op_pool.tile([P, 2 * CS], bf16, name="ot", tag="o")
                nc.vector.tensor_add(ot[:, 0:CS], pp0, obuf[tt][:, 0:CS])
                nc.vector.tensor_add(ot[:, CS:2 * CS], pp1, obuf[tt][:, CS:2 * CS])
                nc.sync.dma_start(outp[tt * P:(tt + 1) * P, :], ot)

        # ---- schedule -------------------------------------------------------
        _drive(qk_pair_gen(0, True), None, 0)
        _drive(attention_gen(0), v_gen(), 1.0)
        _drive(attention_gen(1), qk_pair_gen(1, False), 1.6)
        pa = proj_a_gen()
        _drive(attention_gen(2), pa, 0.7)
        _drive(attention_gen(3), pa, 0.7)
        proj_b()

    nc.compile()
    _BUILT["nc"] = nc
    return nc


def _prep_inputs(x, w_attn, w_proj):
    """Shard + lay out the full inputs for the 8 cores."""
    import ml_dtypes

    bf16 = ml_dtypes.bfloat16
    x = np.asarray(x, dtype=np.float32)
    w_attn = np.asarray(w_attn, dtype=np.float32)
    w_proj = np.asarray(w_proj, dtype=np.float32)

    slopes = _alibi_slopes(H)
    iota = np.arange(T, dtype=np.float32)
    kaug = np.stack([iota, np.ones(T, np.float32)])  # (2, T)
    xTs = [np.ascontiguousarray(x[b].T).astype(bf16) for b in range(B)]

    def ci_major(w):  # (C, HLD) -> (P, CI*HLD)
        return np.ascontiguousarray(
            w.reshape(CI, P, HLD).transpose(1, 0, 2).reshape(P, CI * HLD))

    in_maps = []
    for c in range(N_CORES):
        b, hg = divmod(c, 4)
        heads = [hg, hg + 4, hg + 8, hg + 12]  # slot j gets window WTS[j]
        rows = np.concatenate([np.arange(h * D, (h + 1) * D) for h in heads])
        qaug = np.empty((HL, 2, T), np.float32)
        for j, h in enumerate(heads):
            s = slopes[h]
            qaug[j, 0, :] = s
            qaug[j, 1, :] = -s * iota
        wq = np.ascontiguousarray(w_attn[rows, :].T) * np.float32(0.125)
        wk = np.ascontiguousarray(w_attn[C + rows, :].T)
        wv = np.ascontiguousarray(w_attn[2 * C + rows, :].T)
        wp = np.ascontiguousarray(w_proj[:, rows].T)  # (HLD, C)
        wqk = np.concatenate([ci_major(wq), ci_major(wk)], axis=1).astype(bf16)
        wvp = np.concatenate(
            [ci_major(wv),
             wp.reshape(2, P, C).transpose(1, 0, 2).reshape(P, 2 * C)],
            axis=1).astype(bf16)
        in_maps.append({
            "xT": xTs[b],
            "wqk": wqk,
            "wvp": wvp,
            "kaug": kaug,
            "qaug": qaug,
        })
    return in_maps


def kernel(x, w_attn, w_proj):
    from concourse import bass_utils

    nc = _build()
    in_maps = _prep_inputs(x, w_attn, w_proj)
    res = bass_utils.run_bass_kernel_spmd(nc, in_maps, core_ids=list(range(N_CORES)))
    out = np.zeros((B, T, C), dtype=np.float32)
    for c in range(N_CORES):
        out[c // 4] += np.asarray(res.results[c]["outp"], dtype=np.float32)
    return out
